# revision 1
# baseline (speedup 1.0000x reference)
"""BiLSTM Trainium2 kernel (v2: dual-direction interleaved per core).

out = hf @ out_w[:, :H].T + hb @ out_w[:, H:].T + out_b    (separable)

Sharding (8 cores): each core owns 4 of the 32 batch rows and runs BOTH
direction scans, interleaved step-by-step so one direction's elementwise tail
hides under the other direction's matmul phase. All cores run an identical
program; only the x slice differs per core. Host adds fwd+bwd partials.

Per-core program:
  phase 1 (xproj): xp[b,t,:] = x[b,t,:] @ Wx.T + bias -> DRAM (shared by dirs).
  phase 2 (scan): 512 steps x 2 dirs; per step g = xp_t + h @ Wh.T via
      h.T-stationary [128,BL] x Wh.T-moving [128,512] fp32r matmuls (4 K-chunks
      x 4 gate slices, k-inner so gate psums complete staggered), sigmoid/tanh
      on ACT, cell update on DVE (full-width per gate), h.T built by 4 PE
      transposes into one PSUM bank + 1 DVE f32->f32r copy into a staging ring
      (also next step's stationary), DMA'd to DRAM every 16 steps.
  phase 3 (outproj): out.T[128, T*BL] = w_dir @ h_seq.T per direction.
"""

import sys

sys.path.insert(0, "/opt/trn_rl_repo")

import numpy as np
from contextlib import ExitStack

from concourse import bass, bacc, tile, mybir
from concourse.bass_utils import run_bass_kernel_spmd

F32 = mybir.dt.float32
F32R = mybir.dt.float32r
AF = mybir.ActivationFunctionType

B, T, I, H, O = 32, 512, 256, 512, 128
G = 4 * H          # 2048 gate axis, plain [f | i | o | ch] blocks
BL = B // 8        # 4 batch rows per core
NCORES = 8
# gate slice order per step: f, i, ch, o — heavy cell chain starts early,
# o-gate (needed last) finishes last
SLICE_ORDER = (0, 1, 3, 2)


def _r(ap):
    return ap.bitcast(F32R)


def build_program(n_steps=T, repeats=1, fused=False):
    """Build the per-core Bass program (identical across cores)."""
    assert n_steps % 16 == 0

    nc = bacc.Bacc(
        "TRN2",
        target_bir_lowering=False,
        debug=False,
        num_devices=NCORES,
    )

    rows = n_steps * BL
    xt = nc.dram_tensor("xt", [I, BL * n_steps], F32, kind="ExternalInput").ap()
    wxT = nc.dram_tensor("wxT", [I, G], F32, kind="ExternalInput").ap()
    bx = nc.dram_tensor("bx", [1, G], F32, kind="ExternalInput").ap()
    whT = nc.dram_tensor("whT", [H, G], F32, kind="ExternalInput").ap()
    h0Tb = nc.dram_tensor("h0Tb", [H, BL], F32, kind="ExternalInput").ap()
    c0b = nc.dram_tensor("c0b", [BL, H], F32, kind="ExternalInput").ap()
    wdTf = nc.dram_tensor("wdTf", [H, O], F32, kind="ExternalInput").ap()
    wdTb = nc.dram_tensor("wdTb", [H, O], F32, kind="ExternalInput").ap()
    ob = nc.dram_tensor("ob", [O, 1], F32, kind="ExternalInput").ap()
    ident = nc.dram_tensor("ident", [2 * BL, 2 * BL], F32, kind="ExternalInput").ap()
    outTf = nc.dram_tensor("outTf", [O, rows], F32, kind="ExternalOutput").ap()
    outTb = nc.dram_tensor("outTb", [O, rows], F32, kind="ExternalOutput").ap()

    xp_d = nc.dram_tensor("xp_d", [BL, n_steps, G], F32, kind="Internal").ap()
    hT_d = {
        "f": nc.dram_tensor("hTf_d", [H, n_steps, BL], F32, kind="Internal").ap(),
        "b": nc.dram_tensor("hTb_d", [H, n_steps, BL], F32, kind="Internal").ap(),
    }

    with tile.TileContext(nc) as tc, ExitStack() as ctx:
        const = ctx.enter_context(tc.tile_pool(name="const", bufs=1))
        ps_pool = ctx.enter_context(tc.tile_pool(name="ps", bufs=6, space="PSUM"))
        psT_pool = ctx.enter_context(tc.tile_pool(name="psT", bufs=2, space="PSUM"))
        xp_pool = ctx.enter_context(tc.tile_pool(name="xp", bufs=2))
        stg_pool = ctx.enter_context(tc.tile_pool(name="stg", bufs=4))
        g_pool = ctx.enter_context(tc.tile_pool(name="g", bufs=4))
        act_pool = ctx.enter_context(tc.tile_pool(name="act", bufs=8))
        tmp_pool = ctx.enter_context(tc.tile_pool(name="tmp", bufs=3))
        rhs_pool = ctx.enter_context(tc.tile_pool(name="rhs", bufs=3))
        osb_pool = ctx.enter_context(tc.tile_pool(name="osb", bufs=2))

        # ---- constants ----
        xsb = const.tile([128, 2, BL * n_steps], F32R)
        for c in range(2):
            nc.sync.dma_start(xsb[:, c, :], _r(xt[c * 128:(c + 1) * 128, :]))
        wxT_sb = const.tile([128, 2, G], F32R)
        for c in range(2):
            nc.sync.dma_start(wxT_sb[:, c, :], _r(wxT[c * 128:(c + 1) * 128, :]))
        whT_sb = const.tile([128, 4, G], F32R)
        for c in range(4):
            nc.sync.dma_start(whT_sb[:, c, :], _r(whT[c * 128:(c + 1) * 128, :]))
        bx_sb = const.tile([1, G], F32R)
        nc.sync.dma_start(bx_sb[:], _r(bx[:]))
        ones_f = const.tile([1, 128], F32)
        nc.gpsimd.memset(ones_f[:], 1.0)
        ones_sb = const.tile([1, 128], F32R)
        nc.vector.tensor_copy(ones_sb[:], ones_f[:])
        h0T_sb = {}
        h0T_sb["b"] = const.tile([128, 4, BL], F32R, name="h0Tb_sb")
        for c in range(4):
            nc.sync.dma_start(h0T_sb["b"][:, c, :], _r(h0Tb[c * 128:(c + 1) * 128, :]))
        zsf = const.tile([128, 4 * BL], F32)
        nc.gpsimd.memset(zsf[:], 0.0)
        h0T_sb["f"] = const.tile([128, 4, BL], F32R, name="h0Tf_sb")
        nc.vector.tensor_copy(h0T_sb["f"][:, :, :], zsf[:])
        wdT_sb = {}
        for d, src in (("f", wdTf), ("b", wdTb)):
            wdT_sb[d] = const.tile([128, 4, O], F32R, name=f"wdT{d}_sb")
            for c in range(4):
                nc.sync.dma_start(wdT_sb[d][:, c, :], _r(src[c * 128:(c + 1) * 128, :]))
        ob_sb = const.tile([O, 1], F32)
        nc.sync.dma_start(ob_sb[:], ob[:])
        id_sb = const.tile([2 * BL, 2 * BL], F32)
        nc.sync.dma_start(id_sb[:], ident[:])
        zb = const.tile([128, 1], F32)
        nc.gpsimd.memset(zb[:], 0.0)

        # persistent state (rows BL..31 stay zero)
        c_sb = {d: const.tile([32, H], F32, name=f"c{d}_sb") for d in "fb"}
        h_sb = {d: const.tile([32, H], F32, name=f"h{d}_sb") for d in "fb"}
        for d in "fb":
            nc.gpsimd.memset(c_sb[d][:], 0.0)
            nc.gpsimd.memset(h_sb[d][:], 0.0)

        if fused:
            # fused stationary init [zeros(fwd) | bh0(bwd)] as f32r
            z2 = const.tile([128, 4, 2 * BL], F32)
            nc.gpsimd.memset(z2[:], 0.0)
            for c in range(4):
                nc.sync.dma_start(
                    z2[:, c, BL:2 * BL], h0Tb[c * 128:(c + 1) * 128, :]
                )
            h0TF_sb = const.tile([128, 4, 2 * BL], F32R)
            nc.vector.tensor_copy(h0TF_sb[:, :, :], z2[:, :, :])
            cF_sb = const.tile([32, H], F32, name="cF_sb")
            hF_sb = const.tile([32, H], F32, name="hF_sb")
            nc.gpsimd.memset(cF_sb[:], 0.0)
            nc.gpsimd.memset(hF_sb[:], 0.0)
            for _rep in range(repeats):
                _phases_fused(
                    nc, tc, n_steps, xsb, wxT_sb, whT_sb, bx_sb, ones_sb,
                    h0TF_sb, wdT_sb, ob_sb, id_sb, zb, cF_sb, hF_sb, c0b,
                    xp_d, hT_d, outTf, outTb, ps_pool, psT_pool, xp_pool,
                    stg_pool, g_pool, act_pool, tmp_pool, rhs_pool, osb_pool,
                )
        else:
            for _rep in range(repeats):
                _phases(
                    nc, tc, n_steps, xsb, wxT_sb, whT_sb, bx_sb, ones_sb, h0T_sb,
                    wdT_sb, ob_sb, id_sb, zb, c_sb, h_sb, c0b, xp_d, hT_d,
                    outTf, outTb, ps_pool, psT_pool, xp_pool, stg_pool, g_pool,
                    act_pool, tmp_pool, rhs_pool, osb_pool,
                )

    nc.compile()
    return nc


def _phases(
    nc, tc, n_steps, xsb, wxT_sb, whT_sb, bx_sb, ones_sb, h0T_sb,
    wdT_sb, ob_sb, id_sb, zb, c_sb, h_sb, c0b, xp_d, hT_d,
    outTf, outTb, ps_pool, psT_pool, xp_pool, stg_pool, g_pool,
    act_pool, tmp_pool, rhs_pool, osb_pool,
):
    nblk = n_steps // 16
    rows = n_steps * BL

    # per-repeat cell-state init (fwd zero, bwd learned)
    nc.gpsimd.memset(c_sb["f"][0:BL, :], 0.0)
    nc.sync.dma_start(c_sb["b"][0:BL, :], c0b[:])

    # ---- phase 1: xproj (shared by both directions) ----
    nrowblk = (BL * n_steps) // 128
    for j in range(nrowblk):
        for s in range(4):
            ps = ps_pool.tile([128, 512], F32, tag="ps", name=f"xps{j}_{s}")
            for c in range(2):
                nc.tensor.matmul(
                    ps[:],
                    xsb[:, c, j * 128:(j + 1) * 128],
                    wxT_sb[:, c, s * 512:(s + 1) * 512],
                    start=(c == 0),
                    stop=False,
                )
            nc.tensor.matmul(
                ps[:],
                ones_sb[0:1, 0:128],
                bx_sb[0:1, s * 512:(s + 1) * 512],
                start=False,
                stop=True,
            )
            xq = osb_pool.tile([128, 512], F32, tag="xq", name=f"xq{j}_{s}")
            nc.vector.tensor_copy(xq[:], ps[:])
            nc.sync.dma_start(
                xp_d.flatten_outer_dims()[
                    j * 128:(j + 1) * 128, s * 512:(s + 1) * 512
                ],
                xq[:],
            )

    # ---- phase 2: interleaved dual-direction scan ----
    prev_stg = {"f": None, "b": None}
    for blk in range(nblk):
        stg = {
            d: stg_pool.tile([128, 4, 16, BL], F32R, tag=f"stg{d}",
                             name=f"stg{d}_{blk}")
            for d in "fb"
        }
        for tt in range(16):
            t = blk * 16 + tt
            for d in "fb":
                td = t if d == "f" else n_steps - 1 - t
                xpt = xp_pool.tile([BL, G], F32, tag=f"xp{d}", name=f"xp{d}_{t}")
                nc.sync.dma_start(xpt[:], xp_d[:, td, :])
                gs = {}
                tc2 = None
                for gate in SLICE_ORDER:
                    ps = ps_pool.tile([BL, 512], F32, tag="ps",
                                      name=f"ps{d}_{t}_{gate}")
                    for k in range(4):
                        if t == 0:
                            lhsT = h0T_sb[d][:, k, :]
                        elif tt == 0:
                            lhsT = prev_stg[d][:, k, 15, :]
                        else:
                            lhsT = stg[d][:, k, tt - 1, :]
                        nc.tensor.matmul(
                            ps[:],
                            lhsT,
                            whT_sb[:, k, gate * 512:(gate + 1) * 512],
                            start=(k == 0),
                            stop=(k == 3),
                        )
                    g = g_pool.tile([BL, 512], F32, tag="g",
                                    name=f"g{d}_{t}_{gate}")
                    nc.vector.tensor_add(
                        g[:], ps[:], xpt[:, gate * 512:(gate + 1) * 512]
                    )
                    a = act_pool.tile([BL, 512], F32, tag="a",
                                      name=f"a{d}_{t}_{gate}")
                    nc.scalar.activation(
                        a[:], g[:],
                        AF.Tanh if gate == 3 else AF.Sigmoid,
                        bias=zb[0:BL, 0:1],
                    )
                    gs[gate] = a
                    if gate == 0:          # cm = f * c_prev (early)
                        cm = tmp_pool.tile([BL, H], F32, tag="cm",
                                           name=f"cm{d}_{t}")
                        nc.vector.tensor_mul(cm[:], a[:], c_sb[d][0:BL, :])
                    elif gate == 3:        # c = cm + i*ch ; tanh(c)
                        ic = tmp_pool.tile([BL, H], F32, tag="ic",
                                           name=f"ic{d}_{t}")
                        nc.vector.tensor_mul(ic[:], gs[1][:], a[:])
                        nc.vector.tensor_add(c_sb[d][0:BL, :], cm[:], ic[:])
                        tc2 = tmp_pool.tile([BL, H], F32, tag="tc",
                                            name=f"tc{d}_{t}")
                        nc.scalar.activation(
                            tc2[:], c_sb[d][0:BL, :], AF.Tanh, bias=zb[0:BL, 0:1]
                        )
                    elif gate == 2:        # h = o * tanh(c)
                        nc.vector.tensor_mul(h_sb[d][0:BL, :], a[:], tc2[:])
                # h.T via 4 PE transposes into one PSUM bank, then 1 f32r copy
                pst = psT_pool.tile([128, 4 * BL], F32, tag="pst",
                                    name=f"pst{d}_{t}")
                for c in range(4):
                    nc.tensor.transpose(
                        pst[:, c * BL:(c + 1) * BL],
                        h_sb[d][0:BL, c * 128:(c + 1) * 128],
                        id_sb[0:BL, 0:BL],
                    )
                nc.vector.tensor_copy(stg[d][:, :, tt, :], pst[:])
        for d in "fb":
            dst = hT_d[d]
            for c in range(4):
                nc.sync.dma_start(
                    _r(dst[c * 128:(c + 1) * 128, blk * 16:(blk + 1) * 16, :]),
                    stg[d][:, c, :, :],
                )
            prev_stg[d] = stg[d]

    # ---- phase 3: output projections ----
    nblk_sz = min(512, rows)
    nrb = rows // nblk_sz
    for d, outdst, bias in (("f", outTf, ob_sb), ("b", outTb, zb)):
        for half in range(max(1, (nrb + 3) // 4)):
            rbs = list(range(half * 4, min(nrb, half * 4 + 4)))
            pss = {}
            for k in range(4):
                for rb in rbs:
                    if k == 0:
                        pss[rb] = ps_pool.tile(
                            [O, nblk_sz], F32, tag="ps", name=f"ops{d}{rb}"
                        )
                    rhs = rhs_pool.tile([128, nblk_sz], F32R, tag="rhs",
                                        name=f"orhs{d}{k}_{rb}")
                    t0 = rb * nblk_sz // BL
                    nc.sync.dma_start(
                        rhs[:],
                        _r(hT_d[d][k * 128:(k + 1) * 128,
                                   t0:t0 + nblk_sz // BL, :]),
                    )
                    nc.tensor.matmul(
                        pss[rb][:],
                        wdT_sb[d][:, k, :],
                        rhs[:],
                        start=(k == 0),
                        stop=(k == 3),
                    )
            for rb in rbs:
                osb = osb_pool.tile([O, nblk_sz], F32, tag="osb",
                                    name=f"osb{d}{rb}")
                nc.scalar.activation(
                    osb[:], pss[rb][:], AF.Identity, bias=bias[0:O, 0:1]
                )
                nc.sync.dma_start(
                    outdst[:, rb * nblk_sz:(rb + 1) * nblk_sz], osb[:]
                )


def host_prepare(inputs, n_steps=T):
    """Build the 8 per-core input maps (identical weights, per-core x slice)."""
    x = np.asarray(inputs["x"], np.float32)
    W = np.concatenate(
        [inputs["Wf_w"], inputs["Wi_w"], inputs["Wo_w"], inputs["Wc_w"]], axis=0
    ).astype(np.float32)
    b = np.concatenate(
        [inputs["Wf_b"], inputs["Wi_b"], inputs["Wo_b"], inputs["Wc_b"]]
    ).astype(np.float32)
    wxT = np.ascontiguousarray(W[:, :I].T)      # [I, G]
    whT = np.ascontiguousarray(W[:, I:].T)      # [H, G]
    out_w = np.asarray(inputs["out_w"], np.float32)
    out_b = np.asarray(inputs["out_b"], np.float32)
    bh0 = np.asarray(inputs["bh0"], np.float32)
    bc0 = np.asarray(inputs["bc0"], np.float32)

    shared = {
        "wxT": wxT,
        "bx": b.reshape(1, G),
        "whT": whT,
        "h0Tb": np.ascontiguousarray(np.repeat(bh0.reshape(H, 1), BL, axis=1)),
        "c0b": np.ascontiguousarray(np.repeat(bc0.reshape(1, H), BL, axis=0)),
        "wdTf": np.ascontiguousarray(out_w[:, :H].T),
        "wdTb": np.ascontiguousarray(out_w[:, H:].T),
        "ob": out_b.reshape(O, 1),
        "ident": np.eye(2 * BL, dtype=np.float32),
    }
    in_maps = []
    for core in range(NCORES):
        xc = x[core * BL:(core + 1) * BL, :n_steps]          # [BL, T, I]
        xtc = np.ascontiguousarray(xc.transpose(2, 0, 1).reshape(I, BL * n_steps))
        in_maps.append({"xt": xtc, **shared})
    return in_maps


def host_gather(results, n_steps=T):
    """Combine per-core outTf/outTb partials into [B, T, O]."""
    out = np.zeros((B, n_steps, O), np.float32)
    for core in range(NCORES):
        af = results[core]["outTf"].reshape(O, n_steps, BL)
        ab = results[core]["outTb"].reshape(O, n_steps, BL)[:, ::-1]
        out[core * BL:(core + 1) * BL] = (af + ab).transpose(2, 1, 0)
    return out


_CACHE = {}


def kernel(**inputs):
    if "nc" not in _CACHE:
        _CACHE["nc"] = build_program(T)
    nc = _CACHE["nc"]
    in_maps = host_prepare(inputs, T)
    res = run_bass_kernel_spmd(nc, in_maps, list(range(NCORES)))
    _CACHE["last_exec_time_ns"] = res.exec_time_ns
    return host_gather(res.results, T)


def run_timed(nc, in_maps, iters=5):
    """Execute the SPMD kernel with device-resident inputs, timing each call."""
    import time as _time
    import jax
    from jax.sharding import Mesh, PartitionSpec, NamedSharding
    from jax.experimental.shard_map import shard_map
    from concourse import bass2jax, mybir as _mb

    bass2jax.install_neuronx_cc_hook()
    n_cores = len(in_maps)

    part_name = nc.partition_id_tensor.name if nc.partition_id_tensor else None
    in_names, out_names, out_avals, zero_outs = [], [], [], []
    for alloc in nc.m.functions[0].allocations:
        if not isinstance(alloc, _mb.MemoryLocationSet):
            continue
        name = alloc.memorylocations[0].name
        if alloc.kind == "ExternalInput":
            if name != part_name:
                in_names.append(name)
        elif alloc.kind == "ExternalOutput":
            out_names.append(name)
            shape = tuple(alloc.tensor_shape)
            dtype = _mb.dt.np(alloc.dtype)
            out_avals.append(jax.core.ShapedArray(shape, dtype))
            zero_outs.append(np.zeros(shape, dtype))
    n_params = len(in_names)
    all_names = in_names + out_names
    if part_name is not None:
        all_names = all_names + [part_name]

    def _body(*args):
        operands = list(args)
        if part_name is not None:
            operands.append(bass2jax.partition_id_tensor())
        outs = bass2jax._bass_exec_p.bind(
            *operands,
            out_avals=tuple(out_avals),
            in_names=tuple(all_names),
            out_names=tuple(out_names),
            lowering_input_output_aliases=(),
            sim_require_finite=True,
            sim_require_nnan=True,
            nc=nc,
        )
        return tuple(outs)

    devices = jax.devices()[:n_cores]
    mesh = Mesh(np.asarray(devices), ("core",))
    spec = PartitionSpec("core")
    nin = n_params + len(out_names)
    fn = jax.jit(
        shard_map(
            _body,
            mesh=mesh,
            in_specs=(spec,) * nin,
            out_specs=(spec,) * len(out_names),
            check_rep=False,
        ),
        keep_unused=True,
    )
    concat_in = [
        np.concatenate([np.asarray(in_maps[c][nm]) for c in range(n_cores)], axis=0)
        for nm in in_names
    ] + [np.zeros((n_cores * z.shape[0], *z.shape[1:]), z.dtype) for z in zero_outs]
    sharding = NamedSharding(mesh, spec)
    dev_in = [jax.device_put(a, sharding) for a in concat_in]
    out = jax.block_until_ready(fn(*dev_in))
    times = []
    for _ in range(iters):
        t0 = _time.perf_counter()
        out = jax.block_until_ready(fn(*dev_in))
        times.append(_time.perf_counter() - t0)
    results = [
        {
            nm: np.asarray(out[i]).reshape(n_cores, *out_avals[i].shape)[c]
            for i, nm in enumerate(out_names)
        }
        for c in range(n_cores)
    ]
    return results, times


def _phases_fused(
    nc, tc, n_steps, xsb, wxT_sb, whT_sb, bx_sb, ones_sb, h0TF_sb,
    wdT_sb, ob_sb, id_sb, zb, cF_sb, hF_sb, c0b, xp_d, hT_d,
    outTf, outTb, ps_pool, psT_pool, xp_pool, stg_pool, g_pool,
    act_pool, tmp_pool, rhs_pool, osb_pool,
):
    """Both directions share one matmul stream: stationary [hfT|hbT] [128, 8].

    State rows 0:BL = fwd, BL:2BL = bwd. Halves PE columns per step; the
    (partly exposed) tail is amortized by gate-staggered psum completion.
    """
    nblk = n_steps // 16
    rows = n_steps * BL
    BW = 2 * BL

    nc.gpsimd.memset(cF_sb[0:BL, :], 0.0)
    nc.sync.dma_start(cF_sb[BL:BW, :], c0b[:])

    # ---- phase 1: xproj (identical to non-fused) ----
    nrowblk = (BL * n_steps) // 128
    for j in range(nrowblk):
        for s in range(4):
            ps = ps_pool.tile([128, 512], F32, tag="ps", name=f"xps{j}_{s}")
            for c in range(2):
                nc.tensor.matmul(
                    ps[:],
                    xsb[:, c, j * 128:(j + 1) * 128],
                    wxT_sb[:, c, s * 512:(s + 1) * 512],
                    start=(c == 0),
                    stop=False,
                )
            nc.tensor.matmul(
                ps[:],
                ones_sb[0:1, 0:128],
                bx_sb[0:1, s * 512:(s + 1) * 512],
                start=False,
                stop=True,
            )
            xq = osb_pool.tile([128, 512], F32, tag="xq", name=f"xq{j}_{s}")
            nc.vector.tensor_copy(xq[:], ps[:])
            nc.sync.dma_start(
                xp_d.flatten_outer_dims()[
                    j * 128:(j + 1) * 128, s * 512:(s + 1) * 512
                ],
                xq[:],
            )

    # ---- phase 2: fused scan ----
    prev_stg = None
    for blk in range(nblk):
        stg = stg_pool.tile([128, 4, 16, BW], F32R, tag="stg",
                            name=f"stg_{blk}")
        for tt in range(16):
            t = blk * 16 + tt
            xpt = xp_pool.tile([BW, G], F32, tag="xp", name=f"xp_{t}")
            nc.sync.dma_start(xpt[0:BL, :], xp_d[:, t, :])
            nc.sync.dma_start(xpt[BL:BW, :], xp_d[:, n_steps - 1 - t, :])
            gs = {}
            tc2 = None
            for gate in SLICE_ORDER:
                ps = ps_pool.tile([BW, 512], F32, tag="ps",
                                  name=f"ps_{t}_{gate}")
                for k in range(4):
                    if t == 0:
                        lhsT = h0TF_sb[:, k, :]
                    elif tt == 0:
                        lhsT = prev_stg[:, k, 15, :]
                    else:
                        lhsT = stg[:, k, tt - 1, :]
                    nc.tensor.matmul(
                        ps[:],
                        lhsT,
                        whT_sb[:, k, gate * 512:(gate + 1) * 512],
                        start=(k == 0),
                        stop=(k == 3),
                    )
                g = g_pool.tile([BW, 512], F32, tag="g", name=f"g_{t}_{gate}")
                nc.vector.tensor_add(
                    g[:], ps[:], xpt[:, gate * 512:(gate + 1) * 512]
                )
                a = act_pool.tile([BW, 512], F32, tag="a", name=f"a_{t}_{gate}")
                nc.scalar.activation(
                    a[:], g[:],
                    AF.Tanh if gate == 3 else AF.Sigmoid,
                    bias=zb[0:BW, 0:1],
                )
                gs[gate] = a
                if gate == 0:
                    cm = tmp_pool.tile([BW, H], F32, tag="cm", name=f"cm_{t}")
                    nc.vector.tensor_mul(cm[:], a[:], cF_sb[0:BW, :])
                elif gate == 3:
                    ic = tmp_pool.tile([BW, H], F32, tag="ic", name=f"ic_{t}")
                    nc.vector.tensor_mul(ic[:], gs[1][:], a[:])
                    nc.vector.tensor_add(cF_sb[0:BW, :], cm[:], ic[:])
                    tc2 = tmp_pool.tile([BW, H], F32, tag="tc", name=f"tc_{t}")
                    nc.scalar.activation(
                        tc2[:], cF_sb[0:BW, :], AF.Tanh, bias=zb[0:BW, 0:1]
                    )
                elif gate == 2:
                    nc.vector.tensor_mul(hF_sb[0:BW, :], a[:], tc2[:])
            pst = psT_pool.tile([128, 4 * BW], F32, tag="pst", name=f"pst_{t}")
            for c in range(4):
                nc.tensor.transpose(
                    pst[:, c * BW:(c + 1) * BW],
                    hF_sb[0:BW, c * 128:(c + 1) * 128],
                    id_sb[:],
                )
            nc.vector.tensor_copy(stg[:, :, tt, :], pst[:])
        for d, lo in (("f", 0), ("b", BL)):
            dst = hT_d[d]
            for c in range(4):
                nc.sync.dma_start(
                    _r(dst[c * 128:(c + 1) * 128, blk * 16:(blk + 1) * 16, :]),
                    stg[:, c, :, lo:lo + BL],
                )
        prev_stg = stg

    # ---- phase 3: output projections (identical to non-fused) ----
    nblk_sz = min(512, rows)
    nrb = rows // nblk_sz
    for d, outdst, bias in (("f", outTf, ob_sb), ("b", outTb, zb)):
        for half in range(max(1, (nrb + 3) // 4)):
            rbs = list(range(half * 4, min(nrb, half * 4 + 4)))
            pss = {}
            for k in range(4):
                for rb in rbs:
                    if k == 0:
                        pss[rb] = ps_pool.tile(
                            [O, nblk_sz], F32, tag="ps", name=f"Fops{d}{rb}"
                        )
                    rhs = rhs_pool.tile([128, nblk_sz], F32R, tag="rhs",
                                        name=f"Forhs{d}{k}_{rb}")
                    t0 = rb * nblk_sz // BL
                    nc.sync.dma_start(
                        rhs[:],
                        _r(hT_d[d][k * 128:(k + 1) * 128,
                                   t0:t0 + nblk_sz // BL, :]),
                    )
                    nc.tensor.matmul(
                        pss[rb][:],
                        wdT_sb[d][:, k, :],
                        rhs[:],
                        start=(k == 0),
                        stop=(k == 3),
                    )
            for rb in rbs:
                osb = osb_pool.tile([O, nblk_sz], F32, tag="osb",
                                    name=f"Fosb{d}{rb}")
                nc.scalar.activation(
                    osb[:], pss[rb][:], AF.Identity, bias=bias[0:O, 0:1]
                )
                nc.sync.dma_start(
                    outdst[:, rb * nblk_sz:(rb + 1) * nblk_sz], osb[:]
                )



# revision 9
# speedup vs baseline: 3.0512x; 3.0512x over previous
"""BiLSTM Trainium2 kernel (v3: transposed-state scan, both directions fused).

out = hf @ out_w[:, :H].T + hb @ out_w[:, H:].T + out_b    (separable)

Sharding (8 cores): each core owns 4 of the 32 batch rows and runs BOTH
direction scans fused in one instruction stream (state columns
[fwd BL | bwd BL] = 8). All cores run an identical program; only the x
slice differs per core.

Key idea vs v2: on TRN2 a matmul instruction costs ~(moving free size)
cycles regardless of the stationary width. v2 streamed the 4H=2048 gate
columns of Wh.T per step (8192 rows/step/dir). v3 computes the gates in
TRANSPOSED form g.T[4H, BW] = sum_k Wh_block.T @ h.T so the moving operand
is the 8-wide state: 64 matmuls x 8 rows + 32 xp-injection matmuls x 4
rows ~= 600 streamed rows/step for BOTH dirs. The transposed layout also
makes H the partition dim for all elementwise work (free size 32), kills
the per-step PE transposes, and keeps the whole h history in SBUF so the
output projection reads it directly.

Per-core program:
  phase 1 (xproj): xpT[g, t*BL+b] = Wx[g,:] @ x.T + bias -> DRAM (g-chunked
      [128, 16, T, BL]); bias applied via ACT per-partition bias.
  phase 2 (scan): T steps; per step the gate psums [128(g-slice), 4, 8] are
      started by identity-matmul injection of xp (start=True) then
      accumulate 4 H-chunks of Wh.T @ h.T; sigmoid/tanh on ACT straight
      from PSUM; cell update on DVE in transposed layout; h written to a
      ping-pong state tile (next step's moving operand) and copied
      time-aligned into SBUF-resident histf/histb.
  phase 3 (outproj): outT[O, T*BL] = wf.T @ histf + wb.T @ histb + ob,
      fwd+bwd fused on-device (host only transposes).
"""

import sys

sys.path.insert(0, "/opt/trn_rl_repo")

import numpy as np
from contextlib import ExitStack

from concourse import bass, bacc, tile, mybir

F32 = mybir.dt.float32
F32R = mybir.dt.float32r
AF = mybir.ActivationFunctionType

B, T, I, H, O = 32, 512, 256, 512, 128
G = 4 * H          # 2048 gate rows, blocks [f | i | o | ch]
BL = B // 8        # 4 batch rows per core
BW = 2 * BL        # 8 state columns: [fwd | bwd]
NCORES = 8
# gate m-slice starts in execution order: f, i, ch, o (o last so the heavy
# c-chain starts as early as possible; o is only needed at the very end)
GATE_M0 = (0, 4, 12, 8)
M0_F, M0_I, M0_O, M0_CH = 0, 4, 8, 12


def _r(ap):
    return ap.bitcast(F32R)


def build_program(n_steps=T, repeats=1):
    """Build the per-core Bass program (identical across cores)."""
    assert n_steps % 128 == 0

    nc = bacc.Bacc(
        "TRN2",
        target_bir_lowering=False,
        debug=False,
        num_devices=NCORES,
    )

    rows = n_steps * BL
    xt = nc.dram_tensor("xt", [I, rows], F32, kind="ExternalInput").ap()
    wxT = nc.dram_tensor("wxT", [I, G], F32, kind="ExternalInput").ap()
    bxT = nc.dram_tensor("bxT", [G, 1], F32, kind="ExternalInput").ap()
    whT = nc.dram_tensor("whT", [H, G], F32, kind="ExternalInput").ap()
    h0T = nc.dram_tensor("h0T", [H, BL], F32, kind="ExternalInput").ap()
    c0T = nc.dram_tensor("c0T", [H, BL], F32, kind="ExternalInput").ap()
    wdTf = nc.dram_tensor("wdTf", [H, O], F32, kind="ExternalInput").ap()
    wdTb = nc.dram_tensor("wdTb", [H, O], F32, kind="ExternalInput").ap()
    ob = nc.dram_tensor("ob", [O, 1], F32, kind="ExternalInput").ap()
    ident = nc.dram_tensor("ident", [128, 128], F32, kind="ExternalInput").ap()
    outT = nc.dram_tensor("outT", [O, rows], F32, kind="ExternalOutput").ap()

    xpT_d = nc.dram_tensor("xpT_d", [128, 16, n_steps, BL], F32, kind="Internal").ap()

    with tile.TileContext(nc) as tc, ExitStack() as ctx:
        const = ctx.enter_context(tc.tile_pool(name="const", bufs=1))
        bigps = ctx.enter_context(tc.tile_pool(name="bigps", bufs=3, space="PSUM"))
        gps = ctx.enter_context(tc.tile_pool(name="gps", bufs=3, space="PSUM"))
        xp_pool = ctx.enter_context(tc.tile_pool(name="xp", bufs=4))
        act_pool = ctx.enter_context(tc.tile_pool(name="act", bufs=8))
        tmp_pool = ctx.enter_context(tc.tile_pool(name="tmp", bufs=6))
        hs_pool = ctx.enter_context(tc.tile_pool(name="hs", bufs=3))
        osb_pool = ctx.enter_context(tc.tile_pool(name="osb", bufs=3))

        # ---- constants ----
        xsb = const.tile([128, 2, rows], F32R)
        for c in range(2):
            nc.sync.dma_start(xsb[:, c, :], _r(xt[c * 128:(c + 1) * 128, :]))
        wxT_sb = const.tile([128, 2, G], F32R)
        for c in range(2):
            nc.sync.dma_start(wxT_sb[:, c, :], _r(wxT[c * 128:(c + 1) * 128, :]))
        whT_sb = const.tile([128, 4, G], F32R)
        for c in range(4):
            nc.sync.dma_start(whT_sb[:, c, :], _r(whT[c * 128:(c + 1) * 128, :]))
        bxT_sb = const.tile([128, 16], F32)
        for m in range(16):
            nc.sync.dma_start(bxT_sb[:, m:m + 1], bxT[m * 128:(m + 1) * 128, :])
        id_sb = const.tile([128, 128], F32R)
        nc.sync.dma_start(id_sb[:], _r(ident[:]))
        wdT_sb = {}
        for d, src in (("f", wdTf), ("b", wdTb)):
            wdT_sb[d] = const.tile([128, 4, O], F32R, name=f"wdT{d}_sb")
            for c in range(4):
                nc.sync.dma_start(wdT_sb[d][:, c, :], _r(src[c * 128:(c + 1) * 128, :]))
        ob_sb = const.tile([O, 1], F32)
        nc.sync.dma_start(ob_sb[:], ob[:])
        zb = const.tile([128, 1], F32)
        nc.gpsimd.memset(zb[:], 0.0)

        # fused scan init state [zeros(fwd) | learned(bwd)]
        z4 = const.tile([128, 4, BW], F32)
        nc.gpsimd.memset(z4[:], 0.0)
        for k in range(4):
            nc.sync.dma_start(z4[:, k, BL:BW], h0T[k * 128:(k + 1) * 128, :])
        h0TF = const.tile([128, 4, BW], F32R)
        nc.vector.tensor_copy(h0TF[:], z4[:])
        cF = const.tile([128, 4, BW], F32)

        # SBUF-resident hidden history, time-aligned per direction
        histf = const.tile([128, 4, n_steps, BL], F32R, name="histf")
        histb = const.tile([128, 4, n_steps, BL], F32R, name="histb")

        for _rep in range(repeats):
            _phases(
                nc, n_steps, xsb, wxT_sb, whT_sb, bxT_sb, id_sb, wdT_sb,
                ob_sb, zb, h0TF, cF, c0T, histf, histb, xpT_d, outT,
                bigps, gps, xp_pool, act_pool, tmp_pool, hs_pool, osb_pool,
            )

    nc.compile()
    return nc


def _phases(
    nc, n_steps, xsb, wxT_sb, whT_sb, bxT_sb, id_sb, wdT_sb,
    ob_sb, zb, h0TF, cF, c0T, histf, histb, xpT_d, outT,
    bigps, gps, xp_pool, act_pool, tmp_pool, hs_pool, osb_pool,
):
    nblk = n_steps // 16
    rows = n_steps * BL
    ncb = rows // 512

    # per-repeat cell-state init (fwd zero, bwd learned)
    nc.gpsimd.memset(cF[:], 0.0)
    for k in range(4):
        nc.sync.dma_start(cF[:, k, BL:BW], c0T[k * 128:(k + 1) * 128, :])

    # ---- phase 1: transposed xproj ----
    for cb in range(ncb):
        for m in range(16):
            ps = bigps.tile([128, 512], F32, tag="bps", name=f"xps{cb}_{m}")
            for c in range(2):
                nc.tensor.matmul(
                    ps[:],
                    wxT_sb[:, c, m * 128:(m + 1) * 128],
                    xsb[:, c, cb * 512:(cb + 1) * 512],
                    start=(c == 0),
                    stop=(c == 1),
                )
            xq = osb_pool.tile([128, 512], F32, tag="osb", name=f"xq{cb}_{m}")
            nc.scalar.activation(xq[:], ps[:], AF.Identity, bias=bxT_sb[:, m:m + 1])
            nc.sync.dma_start(xpT_d[:, m, cb * 128:(cb + 1) * 128, :], xq[:])

    # ---- phase 2: fused transposed scan ----
    hprev = h0TF
    for blk in range(nblk):
        t0 = blk * 16
        xpf = xp_pool.tile([128, 16, 16, BL], F32R, tag="xpf", name=f"xpf{blk}")
        nc.sync.dma_start(xpf[:], _r(xpT_d[:, :, t0:t0 + 16, :]))
        xpb = xp_pool.tile([128, 16, 16, BL], F32R, tag="xpb", name=f"xpb{blk}")
        nc.sync.dma_start(xpb[:], _r(xpT_d[:, :, n_steps - 16 - t0:n_steps - t0, :]))
        for tt in range(16):
            t = t0 + tt
            # one full PSUM bank per step: the first matmul's start flag
            # lazily zeroes the whole 2KB zero region, the last one stops it
            ps = gps.tile([128, 512], F32, tag="g", name=f"ps{t}")
            # xp injection (no h dependency, so these run during the previous
            # step's elementwise tail)
            first = True
            for m0 in GATE_M0:
                for mm in range(4):
                    m = m0 + mm
                    nc.tensor.matmul(
                        ps[:, m * BW:m * BW + BL], id_sb[:],
                        xpf[:, m, tt, :],
                        start=first, stop=False,
                    )
                    first = False
                    nc.tensor.matmul(
                        ps[:, m * BW + BL:(m + 1) * BW], id_sb[:],
                        xpb[:, m, 15 - tt, :],
                        start=False, stop=False,
                    )
            # recurrent part: moving operand is the 8-wide state
            for m0 in GATE_M0:
                for mm in range(4):
                    m = m0 + mm
                    for k in range(4):
                        nc.tensor.matmul(
                            ps[:, m * BW:(m + 1) * BW],
                            whT_sb[:, k, m * 128:(m + 1) * 128],
                            hprev[:, k, :],
                            start=False,
                            stop=(m0 == GATE_M0[-1] and mm == 3 and k == 3),
                        )
            # activations (o's ACT issued before tanh(c) so it isn't queued
            # behind the c-chain)
            a = {}
            cm = None
            for m0 in GATE_M0:
                at = act_pool.tile([128, 4, BW], F32, tag="a", name=f"a{t}_{m0}")
                nc.scalar.activation(
                    at[:],
                    ps[:, m0 * BW:(m0 + 4) * BW].rearrange(
                        "p (m w) -> p m w", w=BW
                    ),
                    AF.Tanh if m0 == M0_CH else AF.Sigmoid,
                    bias=zb[:, 0:1],
                )
                a[m0] = at
                if m0 == M0_F:
                    cm = tmp_pool.tile([128, 4, BW], F32, tag="cm", name=f"cm{t}")
                    nc.vector.tensor_mul(cm[:], at[:], cF[:])
            ic = tmp_pool.tile([128, 4, BW], F32, tag="ic", name=f"ic{t}")
            nc.vector.tensor_mul(ic[:], a[M0_I][:], a[M0_CH][:])
            nc.vector.tensor_add(cF[:], cm[:], ic[:])
            tc2 = tmp_pool.tile([128, 4, BW], F32, tag="tc", name=f"tc{t}")
            nc.scalar.activation(tc2[:], cF[:], AF.Tanh, bias=zb[:, 0:1])
            h = hs_pool.tile([128, 4, BW], F32R, tag="h", name=f"h{t}")
            nc.vector.tensor_mul(h[:], a[M0_O][:], tc2[:])
            nc.vector.tensor_copy(histf[:, :, t, :], h[:, :, 0:BL])
            nc.vector.tensor_copy(histb[:, :, n_steps - 1 - t, :], h[:, :, BL:BW])
            hprev = h

    # ---- phase 3: output projection, fwd+bwd+bias fused ----
    for cb in range(ncb):
        ps = bigps.tile([128, 512], F32, tag="bps", name=f"ops{cb}")
        step = 0
        for d, hist in (("f", histf), ("b", histb)):
            for k in range(4):
                rhs = hist[:, k].rearrange("p t b -> p (t b)")[
                    :, cb * 512:(cb + 1) * 512
                ]
                nc.tensor.matmul(
                    ps[:], wdT_sb[d][:, k, :], rhs,
                    start=(step == 0), stop=(step == 7),
                )
                step += 1
        osb = osb_pool.tile([128, 512], F32, tag="osb", name=f"osb{cb}")
        nc.scalar.activation(osb[:], ps[:], AF.Identity, bias=ob_sb[:, 0:1])
        nc.sync.dma_start(outT[:, cb * 512:(cb + 1) * 512], osb[:])


def host_prepare(inputs, n_steps=T):
    """Build the 8 per-core input maps (identical weights, per-core x slice)."""
    x = np.asarray(inputs["x"], np.float32)
    W = np.concatenate(
        [inputs["Wf_w"], inputs["Wi_w"], inputs["Wo_w"], inputs["Wc_w"]], axis=0
    ).astype(np.float32)
    b = np.concatenate(
        [inputs["Wf_b"], inputs["Wi_b"], inputs["Wo_b"], inputs["Wc_b"]]
    ).astype(np.float32)
    out_w = np.asarray(inputs["out_w"], np.float32)
    out_b = np.asarray(inputs["out_b"], np.float32)
    bh0 = np.asarray(inputs["bh0"], np.float32).reshape(H, 1)
    bc0 = np.asarray(inputs["bc0"], np.float32).reshape(H, 1)

    shared = {
        "wxT": np.ascontiguousarray(W[:, :I].T),
        "bxT": b.reshape(G, 1),
        "whT": np.ascontiguousarray(W[:, I:].T),
        "h0T": np.ascontiguousarray(np.repeat(bh0, BL, axis=1)),
        "c0T": np.ascontiguousarray(np.repeat(bc0, BL, axis=1)),
        "wdTf": np.ascontiguousarray(out_w[:, :H].T),
        "wdTb": np.ascontiguousarray(out_w[:, H:].T),
        "ob": out_b.reshape(O, 1),
        "ident": np.eye(128, dtype=np.float32),
    }
    in_maps = []
    for core in range(NCORES):
        xc = x[core * BL:(core + 1) * BL, :n_steps]          # [BL, T, I]
        xtc = np.ascontiguousarray(
            xc.transpose(2, 1, 0).reshape(I, n_steps * BL)   # col = t*BL + b
        )
        in_maps.append({"xt": xtc, **shared})
    return in_maps


def host_gather(results, n_steps=T):
    """Combine per-core outT into [B, T, O]."""
    out = np.empty((B, n_steps, O), np.float32)
    for core in range(NCORES):
        a = results[core]["outT"].reshape(O, n_steps, BL)
        out[core * BL:(core + 1) * BL] = a.transpose(2, 1, 0)
    return out


def _make_runner(nc, n_cores=NCORES, chain=1):
    """Build a persistent jitted dispatch fn (one trace, reused across calls).

    chain > 1 executes the same NEFF back-to-back on-device that many times,
    each execution's outputs feeding the next one's output-donation buffers
    (a real data dependency, so XLA cannot elide or reorder them). Used for
    repeat-slope timing; the final result equals a single execution since the
    kernel fully overwrites its outputs.
    """
    import jax
    from jax.sharding import Mesh, PartitionSpec, NamedSharding
    from jax.experimental.shard_map import shard_map
    from concourse import bass2jax, mybir as _mb

    bass2jax.install_neuronx_cc_hook()

    part_name = nc.partition_id_tensor.name if nc.partition_id_tensor else None
    in_names, out_names, out_avals, zero_outs = [], [], [], []
    for alloc in nc.m.functions[0].allocations:
        if not isinstance(alloc, _mb.MemoryLocationSet):
            continue
        name = alloc.memorylocations[0].name
        if alloc.kind == "ExternalInput":
            if name != part_name:
                in_names.append(name)
        elif alloc.kind == "ExternalOutput":
            out_names.append(name)
            shape = tuple(alloc.tensor_shape)
            dtype = _mb.dt.np(alloc.dtype)
            out_avals.append(jax.core.ShapedArray(shape, dtype))
            zero_outs.append(np.zeros(shape, dtype))
    n_params = len(in_names)
    all_names = list(in_names) + out_names
    if part_name is not None:
        all_names = all_names + [part_name]

    def _body(*args):
        params = list(args[:n_params])
        outs = list(args[n_params:])
        for _ in range(chain):
            operands = params + list(outs)
            if part_name is not None:
                operands.append(bass2jax.partition_id_tensor())
            outs = bass2jax._bass_exec_p.bind(
                *operands,
                out_avals=tuple(out_avals),
                in_names=tuple(all_names),
                out_names=tuple(out_names),
                lowering_input_output_aliases=(),
                sim_require_finite=True,
                sim_require_nnan=True,
                nc=nc,
            )
        return tuple(outs)

    devices = jax.devices()[:n_cores]
    mesh = Mesh(np.asarray(devices), ("core",))
    spec = PartitionSpec("core")
    nin = n_params + len(out_names)
    fn = jax.jit(
        shard_map(
            _body,
            mesh=mesh,
            in_specs=(spec,) * nin,
            out_specs=(spec,) * len(out_names),
            check_rep=False,
        ),
        keep_unused=True,
    )
    sharding = NamedSharding(mesh, spec)
    return {
        "fn": fn,
        "in_names": in_names,
        "out_names": out_names,
        "out_avals": out_avals,
        "zero_outs": zero_outs,
        "sharding": sharding,
        "n_cores": n_cores,
    }


def _run_spmd(runner, in_maps):
    import jax

    n_cores = runner["n_cores"]
    concat_in = [
        np.concatenate([np.asarray(in_maps[c][nm]) for c in range(n_cores)], axis=0)
        for nm in runner["in_names"]
    ] + [
        np.zeros((n_cores * z.shape[0], *z.shape[1:]), z.dtype)
        for z in runner["zero_outs"]
    ]
    dev_in = [jax.device_put(a, runner["sharding"]) for a in concat_in]
    out = jax.block_until_ready(runner["fn"](*dev_in))
    return [
        {
            nm: np.asarray(out[i]).reshape(n_cores, *runner["out_avals"][i].shape)[c]
            for i, nm in enumerate(runner["out_names"])
        }
        for c in range(n_cores)
    ]


_CACHE = {}


def kernel(**inputs):
    if "runner" not in _CACHE:
        nc = build_program(T)
        _CACHE["nc"] = nc
        _CACHE["runner"] = _make_runner(nc)
    in_maps = host_prepare(inputs, T)
    results = _run_spmd(_CACHE["runner"], in_maps)
    return host_gather(results, T)


# revision 12
# speedup vs baseline: 12.4854x; 4.0920x over previous
"""BiLSTM Trainium2 kernel (v3c: transposed-state scan, bf16 matmul operands).

Same transposed formulation as v3 (moving operand = the 8-wide fused state,
elementwise in [128, 4, 8] layout), but every matmul operand is bf16 so the
128-wide stationary loads at ~1 cyc/col instead of fp32r's ~4 (the dominant
unmodeled cost that sank the f32r version on real HW). xp is added by DVE
for the ch/i/f gates (off the critical path) and matmul-injected through a
bf16 identity for the o gate so its sigmoid reads PSUM directly.
"""

import sys

sys.path.insert(0, "/opt/trn_rl_repo")

import numpy as np
from contextlib import ExitStack

from concourse import bass, bacc, tile, mybir

F32 = mybir.dt.float32
F32R = mybir.dt.float32r
BF16 = mybir.dt.bfloat16
AF = mybir.ActivationFunctionType

B, T, I, H, O = 32, 512, 256, 512, 128
G = 4 * H          # 2048 gate rows, blocks [f | i | o | ch]
BL = B // 8        # 4 batch rows per core
BW = 2 * BL        # 8 state columns: [fwd | bwd]
NCORES = 8
# gate m-slice starts in execution order: ch, i, f, o (the c-chain inputs
# finish early; o - needed only for the final h - streams last)
GATE_M0 = (12, 4, 0, 8)
M0_F, M0_I, M0_O, M0_CH = 0, 4, 8, 12


def _r(ap):
    return ap.bitcast(F32R)


def _bf16np():
    import ml_dtypes
    return ml_dtypes.bfloat16


def build_program(n_steps=T, repeats=1):
    """Build the per-core Bass program (identical across cores)."""
    assert n_steps % 128 == 0

    nc = bacc.Bacc(
        "TRN2",
        target_bir_lowering=False,
        debug=False,
        num_devices=NCORES,
    )

    rows = n_steps * BL
    xt = nc.dram_tensor("xt", [I, rows], BF16, kind="ExternalInput").ap()
    wxT = nc.dram_tensor("wxT", [I, G], BF16, kind="ExternalInput").ap()
    bxT = nc.dram_tensor("bxT", [G, 1], F32, kind="ExternalInput").ap()
    whT = nc.dram_tensor("whT", [H, G], BF16, kind="ExternalInput").ap()
    h0T = nc.dram_tensor("h0T", [H, BL], F32, kind="ExternalInput").ap()
    c0T = nc.dram_tensor("c0T", [H, BL], F32, kind="ExternalInput").ap()
    wdTf = nc.dram_tensor("wdTf", [H, O], BF16, kind="ExternalInput").ap()
    wdTb = nc.dram_tensor("wdTb", [H, O], BF16, kind="ExternalInput").ap()
    ob = nc.dram_tensor("ob", [O, 1], F32, kind="ExternalInput").ap()
    ident = nc.dram_tensor("ident", [128, 128], BF16, kind="ExternalInput").ap()
    outT = nc.dram_tensor("outT", [O, rows], F32, kind="ExternalOutput").ap()

    xpT_d = nc.dram_tensor("xpT_d", [128, 16, n_steps, BL], BF16, kind="Internal").ap()

    with tile.TileContext(nc) as tc, ExitStack() as ctx:
        const = ctx.enter_context(tc.tile_pool(name="const", bufs=1))
        bigps = ctx.enter_context(tc.tile_pool(name="bigps", bufs=3, space="PSUM"))
        gps = ctx.enter_context(tc.tile_pool(name="gps", bufs=3, space="PSUM"))
        xp_pool = ctx.enter_context(tc.tile_pool(name="xp", bufs=4))
        g_pool = ctx.enter_context(tc.tile_pool(name="g", bufs=8))
        act_pool = ctx.enter_context(tc.tile_pool(name="act", bufs=8))
        tmp_pool = ctx.enter_context(tc.tile_pool(name="tmp", bufs=6))
        hs_pool = ctx.enter_context(tc.tile_pool(name="hs", bufs=3))
        osb_pool = ctx.enter_context(tc.tile_pool(name="osb", bufs=3))

        # ---- constants ----
        xsb = const.tile([128, 2, rows], BF16)
        for c in range(2):
            nc.sync.dma_start(xsb[:, c, :], xt[c * 128:(c + 1) * 128, :])
        wxT_sb = const.tile([128, 2, G], BF16)
        for c in range(2):
            nc.sync.dma_start(wxT_sb[:, c, :], wxT[c * 128:(c + 1) * 128, :])
        whT_sb = const.tile([128, 4, G], BF16)
        for c in range(4):
            nc.sync.dma_start(whT_sb[:, c, :], whT[c * 128:(c + 1) * 128, :])
        bxT_sb = const.tile([128, 16], F32)
        for m in range(16):
            nc.sync.dma_start(bxT_sb[:, m:m + 1], bxT[m * 128:(m + 1) * 128, :])
        id_sb = const.tile([128, 128], BF16)
        nc.sync.dma_start(id_sb[:], ident[:])
        wdT_sb = {}
        for d, src in (("f", wdTf), ("b", wdTb)):
            wdT_sb[d] = const.tile([128, 4, O], BF16, name=f"wdT{d}_sb")
            for c in range(4):
                nc.sync.dma_start(wdT_sb[d][:, c, :], src[c * 128:(c + 1) * 128, :])
        ob_sb = const.tile([O, 1], F32)
        nc.sync.dma_start(ob_sb[:], ob[:])
        zb = const.tile([128, 1], F32)
        nc.gpsimd.memset(zb[:], 0.0)

        # fused scan init state [zeros(fwd) | learned(bwd)]
        z4 = const.tile([128, 4, BW], F32)
        nc.gpsimd.memset(z4[:], 0.0)
        for k in range(4):
            nc.sync.dma_start(z4[:, k, BL:BW], h0T[k * 128:(k + 1) * 128, :])
        h0TF = const.tile([128, 4, BW], BF16)
        nc.vector.tensor_copy(h0TF[:], z4[:])
        cF = const.tile([128, 4, BW], F32)

        # SBUF-resident hidden history, time-aligned per direction
        histf = const.tile([128, 4, n_steps, BL], BF16, name="histf")
        histb = const.tile([128, 4, n_steps, BL], BF16, name="histb")

        for _rep in range(repeats):
            _phases(
                nc, n_steps, xsb, wxT_sb, whT_sb, bxT_sb, id_sb, wdT_sb,
                ob_sb, zb, h0TF, cF, c0T, histf, histb, xpT_d, outT,
                bigps, gps, xp_pool, g_pool, act_pool, tmp_pool, hs_pool,
                osb_pool,
            )

    nc.compile()
    return nc


def _phases(
    nc, n_steps, xsb, wxT_sb, whT_sb, bxT_sb, id_sb, wdT_sb,
    ob_sb, zb, h0TF, cF, c0T, histf, histb, xpT_d, outT,
    bigps, gps, xp_pool, g_pool, act_pool, tmp_pool, hs_pool, osb_pool,
):
    nblk = n_steps // 16
    rows = n_steps * BL
    ncb = rows // 512

    # per-repeat cell-state init (fwd zero, bwd learned)
    nc.gpsimd.memset(cF[:], 0.0)
    for k in range(4):
        nc.sync.dma_start(cF[:, k, BL:BW], c0T[k * 128:(k + 1) * 128, :])

    # ---- phase 1: transposed xproj ----
    for cb in range(ncb):
        for m in range(16):
            ps = bigps.tile([128, 512], F32, tag="bps", name=f"xps{cb}_{m}")
            for c in range(2):
                nc.tensor.matmul(
                    ps[:],
                    wxT_sb[:, c, m * 128:(m + 1) * 128],
                    xsb[:, c, cb * 512:(cb + 1) * 512],
                    start=(c == 0),
                    stop=(c == 1),
                )
            xq = osb_pool.tile([128, 512], BF16, tag="osb", name=f"xq{cb}_{m}")
            nc.scalar.activation(xq[:], ps[:], AF.Identity, bias=bxT_sb[:, m:m + 1])
            nc.sync.dma_start(xpT_d[:, m, cb * 128:(cb + 1) * 128, :], xq[:])

    # ---- phase 2: fused transposed scan ----
    hprev = h0TF
    for blk in range(nblk):
        t0 = blk * 16
        xpf = xp_pool.tile([128, 16, 16, BL], BF16, tag="xpf", name=f"xpf{blk}")
        nc.sync.dma_start(xpf[:], xpT_d[:, :, t0:t0 + 16, :])
        xpb = xp_pool.tile([128, 16, 16, BL], BF16, tag="xpb", name=f"xpb{blk}")
        nc.sync.dma_start(xpb[:], xpT_d[:, :, n_steps - 16 - t0:n_steps - t0, :])
        for tt in range(16):
            t = t0 + tt
            # one full PSUM bank per step; the very first matmul starts the
            # zero region, the last wh matmul stops it
            ps = gps.tile([128, 512], F32, tag="g", name=f"ps{t}")
            # o-gate xp injection (no h dependency: runs during the previous
            # step's tail, and lets sigma(o) read PSUM directly)
            first = True
            for mm in range(4):
                m = M0_O + mm
                nc.tensor.matmul(
                    ps[:, m * BW:m * BW + BL], id_sb[:],
                    xpf[:, m, tt, :],
                    start=first, stop=False,
                )
                first = False
                nc.tensor.matmul(
                    ps[:, m * BW + BL:(m + 1) * BW], id_sb[:],
                    xpb[:, m, 15 - tt, :],
                    start=False, stop=False,
                )
            # recurrent part: moving operand is the 8-wide state
            for m0 in GATE_M0:
                for mm in range(4):
                    m = m0 + mm
                    for k in range(4):
                        nc.tensor.matmul(
                            ps[:, m * BW:(m + 1) * BW],
                            whT_sb[:, k, m * 128:(m + 1) * 128],
                            hprev[:, k, :],
                            start=False,
                            stop=(m0 == GATE_M0[-1] and mm == 3 and k == 3),
                        )

            def ps_gate(m0):
                return ps[:, m0 * BW:(m0 + 4) * BW].rearrange(
                    "p (m w) -> p m w", w=BW
                )

            # ch/i/f: add xp on DVE (off the critical path), sigma from SBUF
            a = {}
            for m0 in (M0_CH, M0_I, M0_F):
                gsb = g_pool.tile([128, 4, BW], F32, tag="gsb", name=f"g{t}_{m0}")
                nc.vector.tensor_add(
                    gsb[:, :, 0:BL], ps_gate(m0)[:, :, 0:BL],
                    xpf[:, m0:m0 + 4, tt, :],
                )
                nc.vector.tensor_add(
                    gsb[:, :, BL:BW], ps_gate(m0)[:, :, BL:BW],
                    xpb[:, m0:m0 + 4, 15 - tt, :],
                )
                at = act_pool.tile([128, 4, BW], F32, tag="a", name=f"a{t}_{m0}")
                nc.scalar.activation(
                    at[:], gsb[:],
                    AF.Tanh if m0 == M0_CH else AF.Sigmoid,
                    bias=zb[:, 0:1],
                )
                a[m0] = at
            # cell update
            ic = tmp_pool.tile([128, 4, BW], F32, tag="ic", name=f"ic{t}")
            nc.vector.tensor_mul(ic[:], a[M0_I][:], a[M0_CH][:])
            cm = tmp_pool.tile([128, 4, BW], F32, tag="cm", name=f"cm{t}")
            nc.vector.tensor_mul(cm[:], a[M0_F][:], cF[:])
            nc.vector.tensor_add(cF[:], cm[:], ic[:])
            tc2 = tmp_pool.tile([128, 4, BW], F32, tag="tc", name=f"tc{t}")
            nc.scalar.activation(tc2[:], cF[:], AF.Tanh, bias=zb[:, 0:1])
            # o: sigma straight from PSUM (xp already injected)
            ao = act_pool.tile([128, 4, BW], F32, tag="a", name=f"a{t}_o")
            nc.scalar.activation(ao[:], ps_gate(M0_O), AF.Sigmoid, bias=zb[:, 0:1])
            h = hs_pool.tile([128, 4, BW], BF16, tag="h", name=f"h{t}")
            nc.vector.tensor_mul(h[:], ao[:], tc2[:])
            nc.vector.tensor_copy(histf[:, :, t, :], h[:, :, 0:BL])
            nc.vector.tensor_copy(histb[:, :, n_steps - 1 - t, :], h[:, :, BL:BW])
            hprev = h

    # ---- phase 3: output projection, fwd+bwd+bias fused ----
    for cb in range(ncb):
        ps = bigps.tile([128, 512], F32, tag="bps", name=f"ops{cb}")
        step = 0
        for d, hist in (("f", histf), ("b", histb)):
            for k in range(4):
                rhs = hist[:, k].rearrange("p t b -> p (t b)")[
                    :, cb * 512:(cb + 1) * 512
                ]
                nc.tensor.matmul(
                    ps[:], wdT_sb[d][:, k, :], rhs,
                    start=(step == 0), stop=(step == 7),
                )
                step += 1
        osb = osb_pool.tile([128, 512], F32, tag="osb", name=f"osb{cb}")
        nc.scalar.activation(osb[:], ps[:], AF.Identity, bias=ob_sb[:, 0:1])
        nc.sync.dma_start(outT[:, cb * 512:(cb + 1) * 512], osb[:])


def host_prepare(inputs, n_steps=T):
    """Build the 8 per-core input maps (identical weights, per-core x slice)."""
    bf16 = _bf16np()
    x = np.asarray(inputs["x"], np.float32)
    W = np.concatenate(
        [inputs["Wf_w"], inputs["Wi_w"], inputs["Wo_w"], inputs["Wc_w"]], axis=0
    ).astype(np.float32)
    b = np.concatenate(
        [inputs["Wf_b"], inputs["Wi_b"], inputs["Wo_b"], inputs["Wc_b"]]
    ).astype(np.float32)
    out_w = np.asarray(inputs["out_w"], np.float32)
    out_b = np.asarray(inputs["out_b"], np.float32)
    bh0 = np.asarray(inputs["bh0"], np.float32).reshape(H, 1)
    bc0 = np.asarray(inputs["bc0"], np.float32).reshape(H, 1)

    shared = {
        "wxT": np.ascontiguousarray(W[:, :I].T).astype(bf16),
        "bxT": b.reshape(G, 1),
        "whT": np.ascontiguousarray(W[:, I:].T).astype(bf16),
        "h0T": np.ascontiguousarray(np.repeat(bh0, BL, axis=1)),
        "c0T": np.ascontiguousarray(np.repeat(bc0, BL, axis=1)),
        "wdTf": np.ascontiguousarray(out_w[:, :H].T).astype(bf16),
        "wdTb": np.ascontiguousarray(out_w[:, H:].T).astype(bf16),
        "ob": out_b.reshape(O, 1),
        "ident": np.eye(128, dtype=np.float32).astype(bf16),
    }
    in_maps = []
    for core in range(NCORES):
        xc = x[core * BL:(core + 1) * BL, :n_steps]          # [BL, T, I]
        xtc = np.ascontiguousarray(
            xc.transpose(2, 1, 0).reshape(I, n_steps * BL)   # col = t*BL + b
        ).astype(bf16)
        in_maps.append({"xt": xtc, **shared})
    return in_maps


def host_gather(results, n_steps=T):
    """Combine per-core outT into [B, T, O]."""
    out = np.empty((B, n_steps, O), np.float32)
    for core in range(NCORES):
        a = results[core]["outT"].reshape(O, n_steps, BL)
        out[core * BL:(core + 1) * BL] = a.transpose(2, 1, 0)
    return out


def _make_runner(nc, n_cores=NCORES):
    """Build a persistent jitted dispatch fn (one trace, reused across calls)."""
    import jax
    from jax.sharding import Mesh, PartitionSpec, NamedSharding
    from jax.experimental.shard_map import shard_map
    from concourse import bass2jax, mybir as _mb

    bass2jax.install_neuronx_cc_hook()

    part_name = nc.partition_id_tensor.name if nc.partition_id_tensor else None
    in_names, out_names, out_avals, zero_outs = [], [], [], []
    for alloc in nc.m.functions[0].allocations:
        if not isinstance(alloc, _mb.MemoryLocationSet):
            continue
        name = alloc.memorylocations[0].name
        if alloc.kind == "ExternalInput":
            if name != part_name:
                in_names.append(name)
        elif alloc.kind == "ExternalOutput":
            out_names.append(name)
            shape = tuple(alloc.tensor_shape)
            dtype = _mb.dt.np(alloc.dtype)
            out_avals.append(jax.core.ShapedArray(shape, dtype))
            zero_outs.append(np.zeros(shape, dtype))
    n_params = len(in_names)
    all_names = list(in_names) + out_names
    if part_name is not None:
        all_names = all_names + [part_name]

    def _body(*args):
        operands = list(args)
        if part_name is not None:
            operands.append(bass2jax.partition_id_tensor())
        outs = bass2jax._bass_exec_p.bind(
            *operands,
            out_avals=tuple(out_avals),
            in_names=tuple(all_names),
            out_names=tuple(out_names),
            lowering_input_output_aliases=(),
            sim_require_finite=True,
            sim_require_nnan=True,
            nc=nc,
        )
        return tuple(outs)

    devices = jax.devices()[:n_cores]
    mesh = Mesh(np.asarray(devices), ("core",))
    spec = PartitionSpec("core")
    nin = n_params + len(out_names)
    fn = jax.jit(
        shard_map(
            _body,
            mesh=mesh,
            in_specs=(spec,) * nin,
            out_specs=(spec,) * len(out_names),
            check_rep=False,
        ),
        keep_unused=True,
    )
    sharding = NamedSharding(mesh, spec)
    return {
        "fn": fn,
        "in_names": in_names,
        "out_names": out_names,
        "out_avals": out_avals,
        "zero_outs": zero_outs,
        "sharding": sharding,
        "n_cores": n_cores,
    }


def _run_spmd(runner, in_maps):
    import jax

    n_cores = runner["n_cores"]
    concat_in = [
        np.concatenate([np.asarray(in_maps[c][nm]) for c in range(n_cores)], axis=0)
        for nm in runner["in_names"]
    ] + [
        np.zeros((n_cores * z.shape[0], *z.shape[1:]), z.dtype)
        for z in runner["zero_outs"]
    ]
    dev_in = [jax.device_put(a, runner["sharding"]) for a in concat_in]
    out = jax.block_until_ready(runner["fn"](*dev_in))
    return [
        {
            nm: np.asarray(out[i]).reshape(n_cores, *runner["out_avals"][i].shape)[c]
            for i, nm in enumerate(runner["out_names"])
        }
        for c in range(n_cores)
    ]


_CACHE = {}


def kernel(**inputs):
    if "runner" not in _CACHE:
        nc = build_program(T)
        _CACHE["nc"] = nc
        _CACHE["runner"] = _make_runner(nc)
    in_maps = host_prepare(inputs, T)
    results = _run_spmd(_CACHE["runner"], in_maps)
    return host_gather(results, T)
